# revision 7
# baseline (speedup 1.0000x reference)
"""AttentionPooling kernel for Trainium2 (8 NeuronCores, SPMD).

Math (reference):
    keys   = x @ Wk.T + bk
    scores = (keys @ query) * scale          # [N]
    attn   = segment_softmax(scores, batch)  # per-graph softmax
    pooled = segment_sum(attn * (x @ Wv.T + bv))
    out    = pooled @ Wo.T + bo

Because softmax weights sum to 1 within each graph, the value/output
projections commute with the pooling:
    out_g = (sum_j attn_gj x_j) @ (Wo Wv).T + (Wo bv + bo)
and the key projection folds into a single vector:
    scores = x @ q2 + const,  q2 = scale * Wk.T @ query
(the constant shift cancels in softmax).  So the device kernel only
computes a segment softmax over x @ q2 and the attn-weighted mean of x;
the tiny [G,128] projection runs on the PE at the end.

Layout: batch is uniform (100 nodes per graph, sorted), so each core gets
625 contiguous graphs = 62500 nodes.  SBUF tiles put GRAPHS on partitions
(125 graphs/tile, 5 tiles/core) with (node_in_graph, hidden) = 12800
elements on the free axis.  All bulk math is bf16 to hit the DVE 2x mode;
reductions use halving trees of dense step-1 adds instead of 1x-mode
InstTensorReduce.
"""

import numpy as np
import ml_dtypes

import concourse.bass as bass
import concourse.bacc as bacc
import concourse.tile as tile
from concourse import mybir

N_CORES = 8
H = 128          # hidden
J = 100          # nodes per graph
G_TOTAL = 5000
N_TOTAL = 500_000
G_CORE = G_TOTAL // N_CORES    # 625
N_CORE = N_TOTAL // N_CORES    # 62500
GP = 125                       # graphs per SBUF tile (partition count)
TILES = G_CORE // GP           # 5
F = J * H                      # free elems per graph = 12800

FP = mybir.dt.float32
BF = mybir.dt.bfloat16

TRACE = False      # test.py sets True to capture an NTFF profile
LAST = {}          # test.py reads exec_time_ns etc. from here
_CACHE = {}


def _build(nc, gp=GP, tiles=TILES):
    """Emit the per-core program.  Identical on all cores; inputs differ."""
    j, h, f = J, H, J * H
    n_core = tiles * gp * j

    x_d = nc.dram_tensor("x", [n_core, h], BF, kind="ExternalInput")
    q2_d = nc.dram_tensor("q2", [1, h], BF, kind="ExternalInput")
    w2t_d = nc.dram_tensor("w2t", [h, h], FP, kind="ExternalInput")
    c2_d = nc.dram_tensor("c2", [h, 1], FP, kind="ExternalInput")
    id_d = nc.dram_tensor("ident", [h, h], FP, kind="ExternalInput")
    out_d = nc.dram_tensor("outT", [h, tiles * gp], FP, kind="ExternalOutput")

    # [tiles, gp, (j h)] view of x: graph-per-partition, contiguous rows
    x_v = x_d[:].rearrange("(t p j) h -> t p (j h)", t=tiles, p=gp, j=j)

    with tile.TileContext(nc) as tc:
        from contextlib import ExitStack

        with ExitStack() as ctx:
            singles = ctx.enter_context(tc.tile_pool(name="singles", bufs=1))
            xpool = ctx.enter_context(tc.tile_pool(name="x", bufs=2))
            big = ctx.enter_context(tc.tile_pool(name="big", bufs=3))
            tree = ctx.enter_context(tc.tile_pool(name="tree", bufs=1))
            small = ctx.enter_context(tc.tile_pool(name="small", bufs=2))
            psum = ctx.enter_context(tc.tile_pool(name="ps", bufs=2, space="PSUM"))
            psum_o = ctx.enter_context(tc.tile_pool(name="pso", bufs=1, space="PSUM"))

            # ---- one-time constants -------------------------------------
            q2_sb = singles.tile([h, h], BF)           # q2 replicated per part
            nc.sync.dma_start(out=q2_sb, in_=q2_d[:].to_broadcast((h, h)))
            w2t_sb = singles.tile([h, h], FP)
            nc.sync.dma_start(out=w2t_sb, in_=w2t_d[:])
            c2_sb = singles.tile([h, 1], FP)
            nc.sync.dma_start(out=c2_sb, in_=c2_d[:])
            id_sb = singles.tile([h, h], FP)
            nc.sync.dma_start(out=id_sb, in_=id_d[:])

            pooled_all = singles.tile([gp, tiles, h], FP)
            poolT = singles.tile([h, tiles * gp], FP)
            outT_sb = singles.tile([h, tiles * gp], FP)

            q2b = q2_sb[:gp, :].unsqueeze(1).broadcast_to((gp, j, h))

            for t in range(tiles):
                x_t = xpool.tile([gp, f], BF, tag="x")
                nc.sync.dma_start(out=x_t, in_=x_v[t])
                x3 = x_t[:].rearrange("p (j h) -> p j h", j=j)

                # ---- scores = x . q2 (multiply + halving tree over h) ---
                xq = big.tile([gp, j, h], BF, tag="big")
                nc.vector.tensor_mul(xq, x3, q2b)

                cur = xq
                w = h
                while w > 2:
                    nxt = tree.tile([gp, j, w // 2], BF, tag=f"t{w // 2}")
                    nc.vector.tensor_add(nxt, cur[:, :, 0 : w // 2],
                                         cur[:, :, w // 2 : w])
                    cur = nxt
                    w //= 2
                scores = tree.tile([gp, j], FP, tag="scores")
                nc.vector.tensor_add(
                    scores[:].unsqueeze(2), cur[:, :, 0:1], cur[:, :, 1:2]
                )

                # ---- segment softmax (per partition = per graph) --------
                negmax = small.tile([gp, 1], FP, tag="negmax")
                nc.vector.tensor_reduce(negmax, scores[:],
                                        axis=mybir.AxisListType.X,
                                        op=mybir.AluOpType.max, negate=True)
                e_bf = small.tile([gp, j], BF, tag="e")
                denom = small.tile([gp, 1], FP, tag="denom")
                nc.scalar.activation(out=e_bf, in_=scores[:],
                                     func=mybir.ActivationFunctionType.Exp,
                                     bias=negmax[:], scale=1.0,
                                     accum_out=denom[:])
                rdenom = small.tile([gp, 1], FP, tag="rdenom")
                nc.vector.reciprocal(rdenom, denom[:])

                # ---- e broadcast along h on the Scalar engine -----------
                e_rep = big.tile([gp, j, h], BF, tag="big")
                nc.scalar.copy(out=e_rep,
                               in_=e_bf[:].unsqueeze(2).broadcast_to((gp, j, h)))

                # ---- weighted x, then halving tree over j ---------------
                xe = big.tile([gp, j, h], BF, tag="big")
                nc.vector.tensor_mul(xe, x3, e_rep[:])

                p50 = tree.tile([gp, 50, h], BF, tag="t64")
                nc.vector.tensor_add(p50, xe[:, 0:50, :], xe[:, 50:100, :])
                p25 = tree.tile([gp, 25, h], BF, tag="t32")
                nc.vector.tensor_add(p25, p50[:, 0:25, :], p50[:, 25:50, :])
                pooled = pooled_all[:, t, :]
                nc.vector.tensor_reduce(
                    pooled, p25[:].transpose([0, 2, 1]),
                    axis=mybir.AxisListType.X, op=mybir.AluOpType.add)
                # normalize by softmax denominator (per-partition scalar)
                nc.vector.tensor_scalar_mul(pooled, in0=pooled, scalar1=rdenom[:])

            # ---- transpose pooled, project, add bias, store -------------
            from contextlib import ExitStack as _ES  # noqa: F401

            for t in range(tiles):
                tp = psum.tile([h, gp], FP, tag="tp")
                nc.tensor.transpose(tp, pooled_all[:, t, :], id_sb[:gp, :gp])
                nc.vector.tensor_copy(poolT[:, t * gp : (t + 1) * gp], tp[:])

            ncols = tiles * gp
            c0 = 0
            while c0 < ncols:
                cw = min(512, ncols - c0)
                po = psum_o.tile([h, cw], FP, tag=f"po{c0}")
                nc.tensor.matmul(po, w2t_sb[:], poolT[:, c0 : c0 + cw])
                nc.scalar.activation(out=outT_sb[:, c0 : c0 + cw], in_=po,
                                     func=mybir.ActivationFunctionType.Identity,
                                     bias=c2_sb[:], scale=1.0)
                c0 += cw
            nc.sync.dma_start(out=out_d[:], in_=outT_sb[:])
    nc.compile()  # bacc passes: register allocation, DCE, nop fusion
    return nc


def _numpy_fallback(x, batch, n_graphs, query, Wk, bk, Wv, bv, Wo, bo):
    """jax segment-op semantics: indices outside [0, G) are dropped, and
    the gather seg[batch] wraps negative indices (numpy does the same)."""
    scale = x.shape[-1] ** -0.5
    keys = x @ Wk.T + bk
    values = x @ Wv.T + bv
    scores = (keys @ query) * scale
    G = int(n_graphs)
    batch = np.asarray(batch, np.int64)
    valid = (batch >= 0) & (batch < G)
    seg_max = np.full(G, -np.inf, np.float32)
    np.maximum.at(seg_max, batch[valid], scores[valid])
    e = np.exp(scores - seg_max[batch])
    denom = np.zeros(G, np.float32)
    np.add.at(denom, batch[valid], e[valid])
    attn = e / denom[batch]
    pooled = np.zeros((G, x.shape[1]), np.float32)
    np.add.at(pooled, batch[valid], attn[valid, None] * values[valid])
    return pooled @ Wo.T + bo


def _ensure_ntff_hook():
    """The axon boot only registers the NTFF profile hook if the image
    ships antenv.axon_hooks; ours doesn't, so inject a shim."""
    try:
        import antenv.axon_hooks  # noqa: F401
        return
    except ImportError:
        pass
    try:
        import sys
        import types

        from trn_agent_boot.trn_boot import _ntff_profile_via_ctypes

        hook = _ntff_profile_via_ctypes("/opt/axon/libaxon_pjrt.so")
        mod = types.ModuleType("antenv.axon_hooks")
        mod._hook = hook
        mod.get_axon_ntff_profile_hook = lambda: mod._hook
        mod.set_axon_ntff_profile_hook = lambda h: setattr(mod, "_hook", h)
        import antenv

        antenv.axon_hooks = mod
        sys.modules["antenv.axon_hooks"] = mod
    except Exception:
        pass


def kernel(x, batch, n_graphs, query, Wk, bk, Wv, bv, Wo, bo):
    x = np.asarray(x, np.float32)
    batch = np.asarray(batch)
    query = np.asarray(query, np.float32)
    Wk, bk = np.asarray(Wk, np.float32), np.asarray(bk, np.float32)
    Wv, bv = np.asarray(Wv, np.float32), np.asarray(bv, np.float32)
    Wo, bo = np.asarray(Wo, np.float32), np.asarray(bo, np.float32)

    n = x.shape[0]
    b64 = np.asarray(batch, np.int64)
    i64 = np.arange(n, dtype=np.int64)
    clean = (i64 * int(n_graphs)) // n
    # jax without x64 computes batch in int32; i*5000 wraps for the last
    # ~70k nodes, which the reference's segment ops then DROP entirely.
    wrapped = (((i64 * int(n_graphs) + 2**31) % 2**32) - 2**31) // n
    quirk = False
    if n == N_TOTAL and int(n_graphs) == G_TOTAL and np.array_equal(b64, wrapped):
        quirk = not np.array_equal(wrapped, clean)
    elif not (n == N_TOTAL and int(n_graphs) == G_TOTAL
              and np.array_equal(b64, clean)):
        return _numpy_fallback(x, batch, n_graphs, query, Wk, bk, Wv, bv,
                               Wo, bo).astype(np.float32)

    scale = np.float32(H) ** np.float32(-0.5)
    q2 = (Wk.T @ query) * scale                     # [H]
    W2 = Wo @ Wv                                    # [H, H]
    c2 = Wo @ bv + bo                               # [H]

    if "nc" not in _CACHE:
        _CACHE["nc"] = _build(
            bacc.Bacc("TRN2", target_bir_lowering=False, debug=False))
    nc = _CACHE["nc"]

    x_bf = x.astype(ml_dtypes.bfloat16)
    q2_bf = np.ascontiguousarray(q2.astype(ml_dtypes.bfloat16)[None, :])
    w2t = np.ascontiguousarray(W2.T.astype(np.float32))
    c2c = np.ascontiguousarray(c2.astype(np.float32)[:, None])
    ident = np.eye(H, dtype=np.float32)

    in_maps = []
    for c in range(N_CORES):
        in_maps.append({
            "x": np.ascontiguousarray(x_bf[c * N_CORE : (c + 1) * N_CORE]),
            "q2": q2_bf, "w2t": w2t, "c2": c2c, "ident": ident,
        })

    if TRACE:
        _ensure_ntff_hook()
    from concourse.bass_utils import run_bass_kernel_spmd
    res = run_bass_kernel_spmd(nc, in_maps, core_ids=list(range(N_CORES)),
                               trace=TRACE)
    LAST["exec_time_ns"] = res.exec_time_ns
    LAST["mean_exec_time_ns"] = res.mean_exec_time_ns
    LAST["trace"] = res.instructions_and_trace

    out = np.empty((G_TOTAL, H), np.float32)
    for c in range(N_CORES):
        out[c * G_CORE : (c + 1) * G_CORE] = res.results[c]["outT"].T

    if quirk:
        # Nodes whose int32 batch went negative were dropped by the
        # reference: graphs past the first-negative node are empty
        # (output exactly bo), and the boundary graph pools only its
        # still-valid nodes.  Recompute that one graph in f32 on host.
        first_neg = int(np.argmax(b64 < 0))
        gb = first_neg // J                    # boundary graph
        out[gb + 1 :] = bo[None, :]
        xs = x[gb * J : first_neg]             # valid nodes of graph gb
        s = xs @ q2
        e = np.exp(s - s.max())
        attn = (e / e.sum()).astype(np.float32)
        out[gb] = (attn @ xs) @ W2.T + c2
    return out
